# revision 1
# baseline (speedup 1.0000x reference)
"""ContactMapLinear Trainium2 kernel, v4.

res = tril((X @ P) @ (Q @ X^T), k=-1), X = features[0, 1:4097, :], 8-core SPMD.

Same sharding as baseline (interleaved seq rows c::8 for the row side,
contiguous col block for the col side). Changes vs baseline:
  - Phase B runs as two m-half passes (inner tiles 0-3 then 4-7) with X col
    tiles kept resident in SBUF; each half AllGathers immediately, so the
    collective starts ~half a phase earlier and hides fully under phase A.
  - Host ships pre-tiled layouts ([128, ko, n] with (ko n) contiguous per
    partition) and k-tiles are loaded in pairs -> 2-4 KB DMA lines.
  - Longer PE warmup (HAM clock ramp) under the initial DMA fill.

  Phase B1: B[:, 0:4, :]  = Q1 @ Xcols_c^T   banks 0-3, AllGather chunk 1
  Phase B2: B[:, 4:8, :]  = Q2 @ Xcols_c^T   banks 4-7, AllGather chunk 2
  Phase A:  AT_c = P^T @ Xrows_c^T           banks 0-7
  Phase C:  staircase S rows = AT^T @ B, strict-lower mask on diag blocks.
"""

import sys

import ml_dtypes
import numpy as np

_TRN_REPO = "/opt/trn_rl_repo"
if _TRN_REPO not in sys.path:
    sys.path.insert(0, _TRN_REPO)

D = 4096          # seq length / feature dim
I = 1024          # inner dim
N_CORES = 8
R = D // N_CORES  # 512 seq rows per core
P = 128           # partitions
KT = D // P       # 32 feature k-tiles
IT = I // P       # 8 inner tiles
MT = R // P       # 4 row m-tiles per core
BF16 = ml_dtypes.bfloat16

_CACHE = {}


def _build(repeat: int = 1, sim: bool = False, bj_bufs: int = 8,
           w_bufs: int = 8, xr_depth: int = 10, oc_bufs: int = 6,
           warmup_mms: int = 12):
    import concourse.mybir as mybir
    import concourse.tile as tile
    from concourse import bacc

    dt = mybir.dt
    mdt = dt.bfloat16
    nc = bacc.Bacc("TRN2", target_bir_lowering=False, debug=False,
                   num_devices=1 if sim else N_CORES)

    # pre-tiled host layouts: partition dim first, (ko, n) contiguous
    xtr_in = nc.declare_dram_parameter("xtr", [P, KT, R], mdt, isOutput=False)
    xtc_in = nc.declare_dram_parameter("xtc", [P, KT, R], mdt, isOutput=False)
    p_in = nc.declare_dram_parameter("p", [P, KT, I], mdt, isOutput=False)
    qt1_in = nc.declare_dram_parameter("qt1", [P, KT, I // 2], mdt,
                                       isOutput=False)
    qt2_in = nc.declare_dram_parameter("qt2", [P, KT, I // 2], mdt,
                                       isOutput=False)
    mask_in = nc.declare_dram_parameter("mask", [P, I], dt.float32,
                                        isOutput=False)
    out = nc.declare_dram_parameter("out", [R, D], dt.float32, isOutput=True)

    out_ap = out.ap().rearrange("(mo mi) n -> mi mo n", mi=P)  # [128, 4, 4096]

    with tile.TileContext(nc) as tc:
        with (
            tc.tile_pool(name="xc", bufs=1) as xc_pool,
            tc.tile_pool(name="xr", bufs=1) as xr_pool,
            tc.tile_pool(name="w", bufs=w_bufs) as w_pool,
            tc.tile_pool(name="ab", bufs=1) as ab_pool,
            tc.tile_pool(name="bj", bufs=bj_bufs) as bj_pool,
            tc.tile_pool(name="oc", bufs=oc_bufs) as oc_pool,
            tc.tile_pool(name="msk", bufs=1) as msk_pool,
            tc.tile_pool(name="ps", bufs=1, space="PSUM") as ps_pool,
            tc.tile_pool(name="dram", bufs=1, space="DRAM") as dram_pool,
        ):
            for _rep in range(repeat):
                if _rep == 0 and warmup_mms:
                    wu = xr_pool.tile([P, R], mdt, name="wu", tag="wu")
                    nc.any.memzero(wu[:])
                    wps = ps_pool.tile([P, R], dt.float32, name="wps",
                                       tag="ps7")
                    for _i in range(warmup_mms):
                        nc.tensor.matmul(wps[:], lhsT=wu[:, :P], rhs=wu[:],
                                         start=(_i == 0),
                                         stop=(_i == warmup_mms - 1))

                at_sb = ab_pool.tile([P, IT, R], mdt, name="at", tag="at")
                b_sb = ab_pool.tile([P, IT, R], mdt, name="b", tag="b")

                # X col tiles: resident across both B passes, loaded in pairs
                xc_sbs = []
                for k2 in range(KT // 2):
                    xc = xc_pool.tile([P, 2, R], mdt, name=f"xc{k2}",
                                      tag=f"xc{k2}")
                    nc.sync.dma_start(out=xc[:], in_=xtc_in.ap()[:, 2 * k2:2 * k2 + 2, :])
                    xc_sbs.append(xc)

                # ---- Phase B (two m-half passes) + chunked AllGather ----
                blocs, balls = [], []
                for half, q_in in ((0, qt1_in), (1, qt2_in)):
                    psums = [
                        ps_pool.tile([P, R], dt.float32, name=f"psb{half}{m}",
                                     tag=f"ps{4 * half + m}")
                        for m in range(4)
                    ]
                    for k2 in range(KT // 2):
                        w_sb = w_pool.tile([P, 2, I // 2], mdt, name="w",
                                           tag="w")
                        nc.sync.dma_start(
                            out=w_sb[:], in_=q_in.ap()[:, 2 * k2:2 * k2 + 2, :])
                        for kk in range(2):
                            k = 2 * k2 + kk
                            for m in range(4):
                                nc.tensor.matmul(
                                    psums[m][:],
                                    lhsT=w_sb[:, kk, m * P:(m + 1) * P],
                                    rhs=xc_sbs[k2][:, kk, :],
                                    start=(k == 0),
                                    stop=(k == KT - 1),
                                )
                    for m in range(4):
                        nc.vector.tensor_copy(out=b_sb[:, 4 * half + m, :],
                                              in_=psums[m][:])
                    bloc = dram_pool.tile([P, 4, R], mdt, name=f"bloc{half}",
                                          tag=f"bloc{half}")
                    ball = dram_pool.tile([N_CORES, P, 4, R], mdt,
                                          name=f"ball{half}",
                                          tag=f"ball{half}",
                                          addr_space="Local" if sim
                                          else "Shared")
                    nc.sync.dma_start(out=bloc[:],
                                      in_=b_sb[:, 4 * half:4 * half + 4, :])
                    if sim:
                        for jj in range(N_CORES):
                            nc.sync.dma_start(out=ball[jj][:1, :1, :],
                                              in_=bloc[:1, :1, :])
                    else:
                        nc.gpsimd.collective_compute(
                            "AllGather",
                            mybir.AluOpType.bypass,
                            replica_groups=[list(range(N_CORES))],
                            ins=[bloc.opt()],
                            outs=[ball.opt()],
                        )
                    blocs.append(bloc)
                    balls.append(ball)

                # ---- Phase A ----
                psums = [
                    ps_pool.tile([P, R], dt.float32, name=f"psa{m}",
                                 tag=f"ps{m}")
                    for m in range(IT)
                ]
                for k2 in range(KT // 2):
                    xr = xr_pool.tile([P, 2, R], mdt,
                                      name=f"xr{k2 % xr_depth}",
                                      tag=f"xr{k2 % xr_depth}")
                    nc.sync.dma_start(out=xr[:],
                                      in_=xtr_in.ap()[:, 2 * k2:2 * k2 + 2, :])
                    w_sb = w_pool.tile([P, 2, I], mdt, name="wp", tag="w")
                    nc.sync.dma_start(out=w_sb[:],
                                      in_=p_in.ap()[:, 2 * k2:2 * k2 + 2, :])
                    for kk in range(2):
                        k = 2 * k2 + kk
                        for m in range(IT):
                            nc.tensor.matmul(
                                psums[m][:],
                                lhsT=w_sb[:, kk, m * P:(m + 1) * P],
                                rhs=xr[:, kk, :],
                                start=(k == 0),
                                stop=(k == KT - 1),
                            )
                for m in range(IT):
                    nc.vector.tensor_copy(out=at_sb[:, m, :], in_=psums[m][:])

                mask_sb = msk_pool.tile([P, I], dt.float32, name="mask",
                                        tag="mask")
                nc.sync.dma_start(out=mask_sb[:], in_=mask_in.ap())

                # ---- Phase C: staircase S rows = AT^T @ B ----
                for j in range(N_CORES):
                    bj = bj_pool.tile([P, IT, R], mdt, name="bj", tag="bj")
                    nc.sync.dma_start(out=bj[:, :4, :], in_=balls[0][j])
                    nc.sync.dma_start(out=bj[:, 4:, :], in_=balls[1][j])
                    for t in range(j // 2, MT):
                        bank = t + 4 * (j % 2)
                        ps = ps_pool.tile([P, R], dt.float32, name=f"psc{t}",
                                          tag=f"ps{bank}")
                        for k in range(IT):
                            nc.tensor.matmul(
                                ps[:],
                                lhsT=at_sb[:, k, t * P:(t + 1) * P],
                                rhs=bj[:, k, :],
                                start=(k == 0),
                                stop=(k == IT - 1),
                            )
                        ot = oc_pool.tile([P, R], dt.float32, name="oc",
                                          tag="oc")
                        if t == j // 2:  # diagonal block: strict-lower mask
                            half = (j % 2) * R
                            nc.vector.tensor_tensor(
                                ot[:], ps[:], mask_sb[:, half:half + R],
                                mybir.AluOpType.mult,
                            )
                        else:
                            nc.vector.tensor_copy(out=ot[:], in_=ps[:])
                        nc.sync.dma_start(out=out_ap[:, t, j * R:(j + 1) * R],
                                          in_=ot[:])

    nc.compile()
    return nc


def _make_in_maps(features: np.ndarray, Pm: np.ndarray, Qm: np.ndarray):
    features = np.asarray(features)
    X = features[0, 1:1 + D, :]
    xt = X.T.astype(BF16)                       # [feat, seq]
    p_bf = np.asarray(Pm).astype(BF16)          # [feat, inner]
    qt_bf = np.asarray(Qm).T.astype(BF16)       # [feat, inner]
    # pre-tile: (ko ki) n -> ki ko n with (ko, n) contiguous
    def pretile(a, n):
        return np.ascontiguousarray(
            a.reshape(KT, P, n).transpose(1, 0, 2))
    p_t = pretile(p_bf, I)
    qt_t = pretile(qt_bf, I)
    qt1 = np.ascontiguousarray(qt_t[:, :, :I // 2])
    qt2 = np.ascontiguousarray(qt_t[:, :, I // 2:])
    r_idx = np.arange(P)
    q_idx = np.arange(I)
    in_maps = []
    for c in range(N_CORES):
        mask_c = (q_idx[None, :] < (8 * r_idx[:, None] + c)).astype(np.float32)
        in_maps.append({
            "xtr": pretile(np.ascontiguousarray(xt[:, c::8]), R),
            "xtc": pretile(np.ascontiguousarray(xt[:, c * R:(c + 1) * R]), R),
            "p": p_t, "qt1": qt1, "qt2": qt2, "mask": mask_c,
        })
    return in_maps


def kernel(features: np.ndarray, P: np.ndarray, Q: np.ndarray) -> np.ndarray:
    from concourse.bass_utils import run_bass_kernel_spmd

    if "nc" not in _CACHE:
        _CACHE["nc"] = _build()
    nc = _CACHE["nc"]

    in_maps = _make_in_maps(features, P, Q)
    res = run_bass_kernel_spmd(nc, in_maps, list(range(N_CORES)))
    out_full = np.empty((D, D), dtype=np.float32)
    for c in range(N_CORES):
        out_full[c::8] = res.results[c]["out"]
    return out_full



# revision 3
# speedup vs baseline: 1.0578x; 1.0578x over previous
"""ContactMapLinear Trainium2 kernel, v4.

res = tril((X @ P) @ (Q @ X^T), k=-1), X = features[0, 1:4097, :], 8-core SPMD.

Same sharding as baseline (interleaved seq rows c::8 for the row side,
contiguous col block for the col side). Changes vs baseline:
  - Phase B runs as two m-half passes (inner tiles 0-3 then 4-7) with X col
    tiles kept resident in SBUF; each half AllGathers immediately, so the
    collective starts ~half a phase earlier and hides fully under phase A.
  - Host ships pre-tiled layouts ([128, ko, n] with (ko n) contiguous per
    partition) and k-tiles are loaded in pairs -> 2-4 KB DMA lines.
  - Longer PE warmup (HAM clock ramp) under the initial DMA fill.

  Phase B1: B[:, 0:4, :]  = Q1 @ Xcols_c^T   banks 0-3, AllGather chunk 1
  Phase B2: B[:, 4:8, :]  = Q2 @ Xcols_c^T   banks 4-7, AllGather chunk 2
  Phase A:  AT_c = P^T @ Xrows_c^T           banks 0-7
  Phase C:  staircase S rows = AT^T @ B, strict-lower mask on diag blocks.
"""

import sys

import ml_dtypes
import numpy as np

_TRN_REPO = "/opt/trn_rl_repo"
if _TRN_REPO not in sys.path:
    sys.path.insert(0, _TRN_REPO)

D = 4096          # seq length / feature dim
I = 1024          # inner dim
N_CORES = 8
R = D // N_CORES  # 512 seq rows per core
P = 128           # partitions
KT = D // P       # 32 feature k-tiles
IT = I // P       # 8 inner tiles
MT = R // P       # 4 row m-tiles per core
BF16 = ml_dtypes.bfloat16

_CACHE = {}


def _build(repeat: int = 1, sim: bool = False, bj_bufs: int = 8,
           w_bufs: int = 8, xr_depth: int = 10, oc_bufs: int = 6,
           warmup_mms: int = 12):
    import concourse.mybir as mybir
    import concourse.tile as tile
    from concourse import bacc

    dt = mybir.dt
    mdt = dt.bfloat16
    nc = bacc.Bacc("TRN2", target_bir_lowering=False, debug=False,
                   num_devices=1 if sim else N_CORES)

    # pre-tiled host layouts: partition dim first, (ko, n) contiguous
    xtr_in = nc.declare_dram_parameter("xtr", [P, KT, R], mdt, isOutput=False)
    xtc_in = nc.declare_dram_parameter("xtc", [P, KT, R], mdt, isOutput=False)
    p_in = nc.declare_dram_parameter("p", [P, KT, I], mdt, isOutput=False)
    qt1_in = nc.declare_dram_parameter("qt1", [P, KT, I // 2], mdt,
                                       isOutput=False)
    qt2_in = nc.declare_dram_parameter("qt2", [P, KT, I // 2], mdt,
                                       isOutput=False)
    mask_in = nc.declare_dram_parameter("mask", [P, I], dt.float32,
                                        isOutput=False)
    out = nc.declare_dram_parameter("out", [R, D], dt.float32, isOutput=True)

    out_ap = out.ap().rearrange("(mo mi) n -> mi mo n", mi=P)  # [128, 4, 4096]

    with tile.TileContext(nc) as tc:
        with (
            tc.tile_pool(name="xc", bufs=1) as xc_pool,
            tc.tile_pool(name="xr", bufs=1) as xr_pool,
            tc.tile_pool(name="w", bufs=w_bufs) as w_pool,
            tc.tile_pool(name="ab", bufs=1) as ab_pool,
            tc.tile_pool(name="bj", bufs=bj_bufs) as bj_pool,
            tc.tile_pool(name="oc", bufs=oc_bufs) as oc_pool,
            tc.tile_pool(name="msk", bufs=1) as msk_pool,
            tc.tile_pool(name="ps", bufs=1, space="PSUM") as ps_pool,
            tc.tile_pool(name="dram", bufs=1, space="DRAM") as dram_pool,
        ):
            for _rep in range(repeat):
                if _rep == 0 and warmup_mms:
                    wu = xr_pool.tile([P, R], mdt, name="wu", tag="wu")
                    nc.any.memzero(wu[:])
                    wps = ps_pool.tile([P, R], dt.float32, name="wps",
                                       tag="ps7")
                    for _i in range(warmup_mms):
                        nc.tensor.matmul(wps[:], lhsT=wu[:, :P], rhs=wu[:],
                                         start=(_i == 0),
                                         stop=(_i == warmup_mms - 1))

                at_sb = ab_pool.tile([P, IT, R], mdt, name="at", tag="at")
                b_sb = ab_pool.tile([P, IT, R], mdt, name="b", tag="b")

                # X col tiles: resident across both B passes, loaded in pairs
                xc_sbs = []
                for k2 in range(KT // 2):
                    xc = xc_pool.tile([P, 2, R], mdt, name=f"xc{k2}",
                                      tag=f"xc{k2}")
                    nc.sync.dma_start(out=xc[:], in_=xtc_in.ap()[:, 2 * k2:2 * k2 + 2, :])
                    xc_sbs.append(xc)

                # ---- Phase B (two m-half passes) + chunked AllGather ----
                blocs, balls = [], []
                for half, q_in in ((0, qt1_in), (1, qt2_in)):
                    psums = [
                        ps_pool.tile([P, R], dt.float32, name=f"psb{half}{m}",
                                     tag=f"ps{4 * half + m}")
                        for m in range(4)
                    ]
                    for k2 in range(KT // 2):
                        w_sb = w_pool.tile([P, 2, I // 2], mdt, name="w",
                                           tag="w")
                        nc.sync.dma_start(
                            out=w_sb[:], in_=q_in.ap()[:, 2 * k2:2 * k2 + 2, :])
                        for kk in range(2):
                            k = 2 * k2 + kk
                            for m in range(4):
                                nc.tensor.matmul(
                                    psums[m][:],
                                    lhsT=w_sb[:, kk, m * P:(m + 1) * P],
                                    rhs=xc_sbs[k2][:, kk, :],
                                    start=(k == 0),
                                    stop=(k == KT - 1),
                                )
                    for m in range(4):
                        nc.vector.tensor_copy(out=b_sb[:, 4 * half + m, :],
                                              in_=psums[m][:])
                    bloc = dram_pool.tile([P, 4, R], mdt, name=f"bloc{half}",
                                          tag=f"bloc{half}")
                    ball = dram_pool.tile([N_CORES, P, 4, R], mdt,
                                          name=f"ball{half}",
                                          tag=f"ball{half}",
                                          addr_space="Local" if sim
                                          else "Shared")
                    nc.sync.dma_start(out=bloc[:],
                                      in_=b_sb[:, 4 * half:4 * half + 4, :])
                    if sim:
                        for jj in range(N_CORES):
                            nc.sync.dma_start(out=ball[jj][:1, :1, :],
                                              in_=bloc[:1, :1, :])
                    else:
                        nc.gpsimd.collective_compute(
                            "AllGather",
                            mybir.AluOpType.bypass,
                            replica_groups=[list(range(N_CORES))],
                            ins=[bloc.opt()],
                            outs=[ball.opt()],
                        )
                    blocs.append(bloc)
                    balls.append(ball)

                # ---- Phase A ----
                psums = [
                    ps_pool.tile([P, R], dt.float32, name=f"psa{m}",
                                 tag=f"ps{m}")
                    for m in range(IT)
                ]
                for k2 in range(KT // 2):
                    xr = xr_pool.tile([P, 2, R], mdt,
                                      name=f"xr{k2 % xr_depth}",
                                      tag=f"xr{k2 % xr_depth}")
                    nc.sync.dma_start(out=xr[:],
                                      in_=xtr_in.ap()[:, 2 * k2:2 * k2 + 2, :])
                    w_sb = w_pool.tile([P, 2, I], mdt, name="wp", tag="w")
                    nc.sync.dma_start(out=w_sb[:],
                                      in_=p_in.ap()[:, 2 * k2:2 * k2 + 2, :])
                    for kk in range(2):
                        k = 2 * k2 + kk
                        for m in range(IT):
                            nc.tensor.matmul(
                                psums[m][:],
                                lhsT=w_sb[:, kk, m * P:(m + 1) * P],
                                rhs=xr[:, kk, :],
                                start=(k == 0),
                                stop=(k == KT - 1),
                            )
                for m in range(IT):
                    nc.vector.tensor_copy(out=at_sb[:, m, :], in_=psums[m][:])

                mask_sb = msk_pool.tile([P, I], dt.float32, name="mask",
                                        tag="mask")
                nc.sync.dma_start(out=mask_sb[:], in_=mask_in.ap())

                # ---- Phase C: staircase S rows = AT^T @ B ----
                for j in range(N_CORES):
                    bj = bj_pool.tile([P, IT, R], mdt, name="bj", tag="bj")
                    nc.sync.dma_start(out=bj[:, :4, :], in_=balls[0][j])
                    nc.sync.dma_start(out=bj[:, 4:, :], in_=balls[1][j])
                    for t in range(j // 2, MT):
                        bank = t + 4 * (j % 2)
                        ps = ps_pool.tile([P, R], dt.float32, name=f"psc{t}",
                                          tag=f"ps{bank}")
                        for k in range(IT):
                            nc.tensor.matmul(
                                ps[:],
                                lhsT=at_sb[:, k, t * P:(t + 1) * P],
                                rhs=bj[:, k, :],
                                start=(k == 0),
                                stop=(k == IT - 1),
                            )
                        ot = oc_pool.tile([P, R], dt.float32, name="oc",
                                          tag="oc")
                        if t == j // 2:  # diagonal block: strict-lower mask
                            half = (j % 2) * R
                            nc.vector.tensor_tensor(
                                ot[:], ps[:], mask_sb[:, half:half + R],
                                mybir.AluOpType.mult,
                            )
                        else:
                            nc.vector.tensor_copy(out=ot[:], in_=ps[:])
                        nc.sync.dma_start(out=out_ap[:, t, j * R:(j + 1) * R],
                                          in_=ot[:])

    nc.compile()
    return nc


def _make_in_maps(features: np.ndarray, Pm: np.ndarray, Qm: np.ndarray):
    features = np.asarray(features)
    X = features[0, 1:1 + D, :]
    xt = X.T.astype(BF16)                       # [feat, seq]
    p_bf = np.asarray(Pm).astype(BF16)          # [feat, inner]
    qt_bf = np.asarray(Qm).T.astype(BF16)       # [feat, inner]
    # pre-tile: (ko ki) n -> ki ko n with (ko, n) contiguous
    def pretile(a, n):
        return np.ascontiguousarray(
            a.reshape(KT, P, n).transpose(1, 0, 2))
    p_t = pretile(p_bf, I)
    qt_t = pretile(qt_bf, I)
    qt1 = np.ascontiguousarray(qt_t[:, :, :I // 2])
    qt2 = np.ascontiguousarray(qt_t[:, :, I // 2:])
    r_idx = np.arange(P)
    q_idx = np.arange(I)
    in_maps = []
    for c in range(N_CORES):
        mask_c = (q_idx[None, :] < (8 * r_idx[:, None] + c)).astype(np.float32)
        in_maps.append({
            "xtr": pretile(np.ascontiguousarray(xt[:, c::8]), R),
            "xtc": pretile(np.ascontiguousarray(xt[:, c * R:(c + 1) * R]), R),
            "p": p_t, "qt1": qt1, "qt2": qt2, "mask": mask_c,
        })
    return in_maps


def kernel(features: np.ndarray, P: np.ndarray, Q: np.ndarray) -> np.ndarray:
    from concourse.bass_utils import run_bass_kernel_spmd

    if "nc" not in _CACHE:
        _CACHE["nc"] = _build()
    nc = _CACHE["nc"]

    in_maps = _make_in_maps(features, P, Q)
    res = run_bass_kernel_spmd(nc, in_maps, list(range(N_CORES)))
    out_full = np.empty((D, D), dtype=np.float32)
    for c in range(N_CORES):
        out_full[c::8] = res.results[c]["out"]
    return out_full
